# revision 10
# baseline (speedup 1.0000x reference)
"""BiDAF attention-flow kernel for Trainium2, data-parallel over batch on 8 cores.

Reference computation (per batch element n):
    s_c = c @ w_c + b_c                       # [CL]
    s_q = q @ w_q + b_q                       # [QL]
    cq  = (c * w_cq) @ q^T + b_cq             # [CL, QL]
    s   = cq + s_c[:, None] + s_q[None, :]    # [CL, QL]
    b_att = softmax(max(s, axis=1))           # [CL]
    q2c  = b_att @ c                          # [D]
    return q2c, s

Device mapping (per core, 2 batch elements):
  - R[d, q] = w_cq[d] * q[n, q, d] + w_c[d] folds the s_c term into the GEMM
    (sum_d c[cl,d] * w_c[d] = s_c[cl]); the s_q + b terms enter via a K=1
    ones-row matmul that pre-fills each PSUM bank.
  - c is DMA-loaded with an inline f32->bf16 cast (SWDGE), transposed on-chip
    by the DMA xbar (one [128, 2048] -> [128, 16, 128] transpose per 8-tile
    group) to give d-major lhsT chunks for the TensorE GEMM.
  - softmax skips the max-subtraction (values are O(10), exp is safe in f32)
    so q2c accumulates per-tile with no global barrier:
    q2c = (sum_cl exp(m_cl) * c[cl,:]) / sum_cl exp(m_cl).
"""

import numpy as np

import concourse.bacc as bacc
import concourse.mybir as mybir
import concourse.tile as tile
from concourse.bass_utils import run_bass_kernel_spmd

F32 = mybir.dt.float32
F32R = mybir.dt.float32r
BF16 = mybir.dt.bfloat16

N_CORES = 8
N_FULL, CL_FULL, QL_FULL, D_FULL = 16, 4096, 64, 256


def build_kernel(N_per=2, CL=4096, QL=64, D=256, GROUP=8):
    """Build the bass program for one core processing N_per batch elements."""
    assert D == 256 and QL == 64
    T = CL // 128            # cl tiles per batch elem
    G = T // GROUP           # tile groups per batch elem
    assert G * GROUP == T

    nc = bacc.Bacc("TRN2", target_bir_lowering=False, debug=False)

    c_d = nc.dram_tensor("c", [N_per, CL, D], F32, kind="ExternalInput")
    q_d = nc.dram_tensor("q", [N_per, QL, D], F32, kind="ExternalInput")
    w_c_d = nc.dram_tensor("w_c", [D], F32, kind="ExternalInput")
    w_q_d = nc.dram_tensor("w_q", [D], F32, kind="ExternalInput")
    w_cq_d = nc.dram_tensor("w_cq", [D], F32, kind="ExternalInput")
    b_c_d = nc.dram_tensor("b_c", [1], F32, kind="ExternalInput")
    b_q_d = nc.dram_tensor("b_q", [1], F32, kind="ExternalInput")
    b_cq_d = nc.dram_tensor("b_cq", [1], F32, kind="ExternalInput")
    ident_d = nc.dram_tensor("ident", [64, 64], F32, kind="ExternalInput")

    s_d = nc.dram_tensor("s_out", [N_per, CL, QL], F32, kind="ExternalOutput")
    q2c_d = nc.dram_tensor("q2c_out", [N_per, D], F32, kind="ExternalOutput")

    with tile.TileContext(nc) as tc:
        with (
            tc.tile_pool(name="const", bufs=1) as const_pool,
            tc.tile_pool(name="qprep", bufs=2) as qprep,
            tc.tile_pool(name="qpsum", bufs=1, space="PSUM") as qpsum,
            tc.tile_pool(name="cb", bufs=6) as cb_pool,
            tc.tile_pool(name="cbT", bufs=6) as cbT_pool,
            tc.tile_pool(name="spsum", bufs=4, space="PSUM") as spsum_pool,
            tc.tile_pool(name="ssb", bufs=4) as ssb_pool,
            tc.tile_pool(name="mg", bufs=3) as m_pool,
            tc.tile_pool(name="upool", bufs=2) as u_pool,
            tc.tile_pool(name="qcpsum", bufs=2, space="PSUM") as qc_pool,
            tc.tile_pool(name="outsb", bufs=2) as out_pool,
        ):
            # ---- constants ----
            ident = const_pool.tile([64, 64], F32)
            nc.scalar.dma_start(ident[:], ident_d.ap())
            ones = const_pool.tile([128, 128], BF16)
            nc.vector.memset(ones[:], 1.0)
            w_c_t = const_pool.tile([128, 2], F32, tag="w_c_t")
            w_cq_t = const_pool.tile([128, 2], F32, tag="w_cq_t")
            for w_t, w_dram in ((w_c_t, w_c_d), (w_cq_t, w_cq_d)):
                nc.scalar.dma_start(w_t[:], w_dram.ap().rearrange("(k p) -> p k", p=128))
            w_q_b = const_pool.tile([128, 2], BF16, tag="w_q_b")
            nc.gpsimd.dma_start(w_q_b[:], w_q_d.ap().rearrange("(k p) -> p k", p=128))
            b3 = const_pool.tile([1, 3], F32, tag="b3")
            for i, b_dram in enumerate((b_c_d, b_q_d, b_cq_d)):
                nc.scalar.dma_start(b3[:, i : i + 1], b_dram.ap().unsqueeze(0))
            bsum = const_pool.tile([1, 1], F32, tag="bsum")
            nc.vector.reduce_sum(bsum[:], b3[:], axis=mybir.AxisListType.X)

            for n in range(N_per):
                # ---- q-side prep: R[d,q] = w_cq[d]*qT[d,q] + w_c[d]; sq_row ----
                q_sb = qprep.tile([QL, D], F32, tag="q_sb")
                nc.scalar.dma_start(q_sb[:], q_d.ap()[n])
                qt_ps = qpsum.tile([128, 2, QL], F32, tag="qt_ps")
                for k in range(2):
                    nc.tensor.transpose(
                        qt_ps[:, k, :], q_sb[:, 128 * k : 128 * (k + 1)], ident[:]
                    )
                qt_b16 = qprep.tile([128, 2, QL], BF16, tag="qt_b16")
                R = qprep.tile([128, 2, QL], BF16, tag="R")
                for k in range(2):
                    nc.scalar.copy(qt_b16[:, k, :], qt_ps[:, k, :])
                    nc.scalar.activation(
                        R[:, k, :],
                        qt_ps[:, k, :],
                        mybir.ActivationFunctionType.Identity,
                        bias=w_c_t[:, k : k + 1],
                        scale=w_cq_t[:, k : k + 1],
                    )
                sq_ps = qpsum.tile([1, QL], F32, tag="sq_ps")
                for k in range(2):
                    nc.tensor.matmul(
                        sq_ps[:],
                        w_q_b[:, k : k + 1],
                        qt_b16[:, k, :],
                        start=(k == 0),
                        stop=(k == 1),
                    )
                sq_row = qprep.tile([1, QL], BF16, tag="sq_row")
                nc.scalar.activation(
                    sq_row[:],
                    sq_ps[:],
                    mybir.ActivationFunctionType.Identity,
                    bias=bsum[:, 0:1],
                )

                U = u_pool.tile([128, T], BF16)
                qc_ps = qc_pool.tile([1, D + T], F32)

                for g in range(G):
                    lo = g * GROUP * 128
                    hi = (g + 1) * GROUP * 128
                    # load 8 cl-tiles, casting f32 -> bf16 inline (SWDGE)
                    cb = cb_pool.tile([128, GROUP, D], BF16)
                    nc.gpsimd.dma_start(
                        cb[:],
                        c_d.ap()[n, lo:hi, :].rearrange("(t p) d -> p t d", p=128),
                    )
                    # xbar transpose: [128cl, (t,d)] -> chunk-major d-on-partition
                    cbT = cbT_pool.tile([128, 2 * GROUP, 128], BF16)
                    nc.sync.dma_start_transpose(
                        cbT[:], cb[:].rearrange("p t d -> p (t d)")
                    )
                    # s = ones^T @ (sq_row + bsum broadcast)  +  c @ R
                    sps = spsum_pool.tile([128, GROUP, QL], F32)
                    nc.tensor.matmul(
                        sps[:],
                        ones[0:1, :],
                        sq_row[:].unsqueeze(1).broadcast_to((1, GROUP, QL)),
                        start=True,
                        stop=False,
                        skip_group_check=True,
                    )
                    for t in range(GROUP):
                        for k in range(2):
                            nc.tensor.matmul(
                                sps[:, t, :],
                                cbT[:, 2 * t + k, :],
                                R[:, k, :],
                                start=False,
                                stop=(k == 1),
                                skip_group_check=True,
                            )
                    s_sb = ssb_pool.tile([128, GROUP, QL], F32)
                    nc.scalar.copy(s_sb[:], sps[:])
                    nc.gpsimd.dma_start(
                        s_d.ap()[n, lo:hi, :].rearrange("(t p) q -> p t q", p=128),
                        s_sb[:],
                    )
                    mg = m_pool.tile([128, GROUP], F32)
                    nc.vector.reduce_max(mg[:], sps[:], axis=mybir.AxisListType.X)
                    nc.scalar.activation(
                        U[:, g * GROUP : (g + 1) * GROUP],
                        mg[:],
                        mybir.ActivationFunctionType.Exp,
                    )
                    for t in range(GROUP):
                        gt = g * GROUP + t
                        nc.tensor.matmul(
                            qc_ps[:, 0:D],
                            U[:, gt : gt + 1],
                            cb[:, t, :],
                            start=(gt == 0),
                            stop=(gt == T - 1),
                            skip_group_check=True,
                        )

                # Z = sum(U) via ones^T @ U then a free-dim reduce; q2c = qc / Z
                nc.tensor.matmul(
                    qc_ps[:, D : D + T],
                    ones[:, 0:1],
                    U[:],
                    start=True,
                    stop=True,
                    skip_group_check=True,
                )
                zz = qprep.tile([1, 1], F32, tag="zz")
                nc.vector.reduce_sum(
                    zz[:], qc_ps[0:1, D : D + T], axis=mybir.AxisListType.X
                )
                rz = qprep.tile([1, 1], F32, tag="rz")
                nc.vector.reciprocal(rz[:], zz[:])
                q2c_sb = out_pool.tile([1, D], F32)
                nc.scalar.activation(
                    q2c_sb[:],
                    qc_ps[0:1, 0:D],
                    mybir.ActivationFunctionType.Copy,
                    bias=0.0,
                    scale=rz[0:1, 0:1],
                )
                nc.scalar.dma_start(q2c_d.ap()[n].unsqueeze(0), q2c_sb[:])

    nc.compile()
    return nc


_NC_CACHE = {}


def _get_nc(key):
    if key not in _NC_CACHE:
        _NC_CACHE[key] = build_kernel(*key)
    return _NC_CACHE[key]


def kernel(c, q, w_c, b_c, w_q, b_q, w_cq, b_cq, **run_kwargs):
    N, CL, D = c.shape
    _, QL, _ = q.shape
    n_per = N // N_CORES
    nc = _get_nc((n_per, CL, QL, D))

    ident = np.eye(64, dtype=np.float32)
    in_maps = []
    for i in range(N_CORES):
        sl = slice(i * n_per, (i + 1) * n_per)
        in_maps.append(
            {
                "c": np.ascontiguousarray(c[sl], dtype=np.float32),
                "q": np.ascontiguousarray(q[sl], dtype=np.float32),
                "w_c": np.asarray(w_c, dtype=np.float32),
                "w_q": np.asarray(w_q, dtype=np.float32),
                "w_cq": np.asarray(w_cq, dtype=np.float32),
                "b_c": np.asarray(b_c, dtype=np.float32),
                "b_q": np.asarray(b_q, dtype=np.float32),
                "b_cq": np.asarray(b_cq, dtype=np.float32),
                "ident": ident,
            }
        )

    res = run_bass_kernel_spmd(
        nc, in_maps, core_ids=list(range(N_CORES)), **run_kwargs
    )
    q2c = np.concatenate([res.results[i]["q2c_out"] for i in range(N_CORES)], axis=0)
    s = np.concatenate([res.results[i]["s_out"] for i in range(N_CORES)], axis=0)
    if run_kwargs:
        return (q2c, s), res
    return q2c, s


# revision 11
# speedup vs baseline: 1.0915x; 1.0915x over previous
"""BiDAF attention-flow kernel for Trainium2, data-parallel over batch on 8 cores.

Reference computation (per batch element n):
    s_c = c @ w_c + b_c                       # [CL]
    s_q = q @ w_q + b_q                       # [QL]
    cq  = (c * w_cq) @ q^T + b_cq             # [CL, QL]
    s   = cq + s_c[:, None] + s_q[None, :]    # [CL, QL]
    b_att = softmax(max(s, axis=1))           # [CL]
    q2c  = b_att @ c                          # [D]
    return q2c, s

Device mapping (per core, 2 batch elements):
  - R[d, q] = w_cq[d] * q[n, q, d] + w_c[d] folds the s_c term into the GEMM
    (sum_d c[cl,d] * w_c[d] = s_c[cl]); the s_q + b terms enter via a K=1
    ones-row matmul that pre-fills each PSUM bank.
  - c is DMA-loaded with an inline f32->bf16 cast (SWDGE), transposed on-chip
    by the DMA xbar (one [128, 2048] -> [128, 16, 128] transpose per 8-tile
    group) to give d-major lhsT chunks for the TensorE GEMM.
  - softmax skips the max-subtraction (values are O(10), exp is safe in f32)
    so q2c accumulates per-tile with no global barrier:
    q2c = (sum_cl exp(m_cl) * c[cl,:]) / sum_cl exp(m_cl).
"""

import numpy as np

import concourse.bacc as bacc
import concourse.mybir as mybir
import concourse.tile as tile
from concourse.bass_utils import run_bass_kernel_spmd

F32 = mybir.dt.float32
F32R = mybir.dt.float32r
BF16 = mybir.dt.bfloat16

N_CORES = 8
N_FULL, CL_FULL, QL_FULL, D_FULL = 16, 4096, 64, 256


def build_kernel(N_per=2, CL=4096, QL=64, D=256, GROUP=8):
    """Build the bass program for one core processing N_per batch elements."""
    assert D == 256 and QL == 64
    T = CL // 128            # cl tiles per batch elem
    G = T // GROUP           # tile groups per batch elem
    assert G * GROUP == T

    nc = bacc.Bacc("TRN2", target_bir_lowering=False, debug=False)

    c_d = nc.dram_tensor("c", [N_per, CL, D], F32, kind="ExternalInput")
    q_d = nc.dram_tensor("q", [N_per, QL, D], F32, kind="ExternalInput")
    w_c_d = nc.dram_tensor("w_c", [D], F32, kind="ExternalInput")
    w_q_d = nc.dram_tensor("w_q", [D], F32, kind="ExternalInput")
    w_cq_d = nc.dram_tensor("w_cq", [D], F32, kind="ExternalInput")
    b_c_d = nc.dram_tensor("b_c", [1], F32, kind="ExternalInput")
    b_q_d = nc.dram_tensor("b_q", [1], F32, kind="ExternalInput")
    b_cq_d = nc.dram_tensor("b_cq", [1], F32, kind="ExternalInput")
    ident_d = nc.dram_tensor("ident", [64, 64], F32, kind="ExternalInput")

    s_d = nc.dram_tensor("s_out", [N_per, CL, QL], F32, kind="ExternalOutput")
    q2c_d = nc.dram_tensor("q2c_out", [N_per, D], F32, kind="ExternalOutput")

    with tile.TileContext(nc) as tc:
        with (
            tc.tile_pool(name="const", bufs=1) as const_pool,
            tc.tile_pool(name="qprep", bufs=2) as qprep,
            tc.tile_pool(name="qpsum", bufs=1, space="PSUM") as qpsum,
            tc.tile_pool(name="cb", bufs=6) as cb_pool,
            tc.tile_pool(name="cbT", bufs=6) as cbT_pool,
            tc.tile_pool(name="spsum", bufs=4, space="PSUM") as spsum_pool,
            tc.tile_pool(name="ssb", bufs=4) as ssb_pool,
            tc.tile_pool(name="mg", bufs=3) as m_pool,
            tc.tile_pool(name="upool", bufs=2) as u_pool,
            tc.tile_pool(name="qcpsum", bufs=2, space="PSUM") as qc_pool,
            tc.tile_pool(name="outsb", bufs=2) as out_pool,
        ):
            # ---- constants ----
            ident = const_pool.tile([64, 64], F32)
            nc.scalar.dma_start(ident[:], ident_d.ap())
            ones = const_pool.tile([128, 128], BF16)
            nc.vector.memset(ones[:], 1.0)
            w_c_t = const_pool.tile([128, 2], F32, tag="w_c_t")
            w_cq_t = const_pool.tile([128, 2], F32, tag="w_cq_t")
            for w_t, w_dram in ((w_c_t, w_c_d), (w_cq_t, w_cq_d)):
                nc.scalar.dma_start(w_t[:], w_dram.ap().rearrange("(k p) -> p k", p=128))
            w_q_b = const_pool.tile([128, 2], BF16, tag="w_q_b")
            nc.gpsimd.dma_start(w_q_b[:], w_q_d.ap().rearrange("(k p) -> p k", p=128))
            b3 = const_pool.tile([1, 3], F32, tag="b3")
            for i, b_dram in enumerate((b_c_d, b_q_d, b_cq_d)):
                nc.scalar.dma_start(b3[:, i : i + 1], b_dram.ap().unsqueeze(0))
            bsum = const_pool.tile([1, 1], F32, tag="bsum")
            nc.vector.reduce_sum(bsum[:], b3[:], axis=mybir.AxisListType.X)

            for n in range(N_per):
                # ---- q-side prep: R[d,q] = w_cq[d]*qT[d,q] + w_c[d]; sq_row ----
                q_sb = qprep.tile([QL, D], F32, tag="q_sb")
                nc.scalar.dma_start(q_sb[:], q_d.ap()[n])
                qt_ps = qpsum.tile([128, 2, QL], F32, tag="qt_ps")
                for k in range(2):
                    nc.tensor.transpose(
                        qt_ps[:, k, :], q_sb[:, 128 * k : 128 * (k + 1)], ident[:]
                    )
                qt_b16 = qprep.tile([128, 2, QL], BF16, tag="qt_b16")
                R = qprep.tile([128, 2, QL], BF16, tag="R")
                for k in range(2):
                    nc.scalar.copy(qt_b16[:, k, :], qt_ps[:, k, :])
                    nc.scalar.activation(
                        R[:, k, :],
                        qt_ps[:, k, :],
                        mybir.ActivationFunctionType.Identity,
                        bias=w_c_t[:, k : k + 1],
                        scale=w_cq_t[:, k : k + 1],
                    )
                sq_ps = qpsum.tile([1, QL], F32, tag="sq_ps")
                for k in range(2):
                    nc.tensor.matmul(
                        sq_ps[:],
                        w_q_b[:, k : k + 1],
                        qt_b16[:, k, :],
                        start=(k == 0),
                        stop=(k == 1),
                    )
                sq_row = qprep.tile([1, QL], BF16, tag="sq_row")
                nc.scalar.activation(
                    sq_row[:],
                    sq_ps[:],
                    mybir.ActivationFunctionType.Identity,
                    bias=bsum[:, 0:1],
                )

                U = u_pool.tile([128, T], BF16)
                qc_ps = qc_pool.tile([1, D + T], F32)

                for g in range(G):
                    lo = g * GROUP * 128
                    hi = (g + 1) * GROUP * 128
                    # load 8 cl-tiles, casting f32 -> bf16 inline (SWDGE)
                    cb = cb_pool.tile([128, GROUP, D], BF16)
                    nc.gpsimd.dma_start(
                        cb[:],
                        c_d.ap()[n, lo:hi, :].rearrange("(t p) d -> p t d", p=128),
                    )
                    # xbar transpose: [128cl, (t,d)] -> chunk-major d-on-partition
                    cbT = cbT_pool.tile([128, 2 * GROUP, 128], BF16)
                    nc.sync.dma_start_transpose(
                        cbT[:], cb[:].rearrange("p t d -> p (t d)")
                    )
                    # s = ones^T @ (sq_row + bsum broadcast)  +  c @ R
                    sps = spsum_pool.tile([128, GROUP, QL], F32)
                    nc.tensor.matmul(
                        sps[:],
                        ones[0:1, :],
                        sq_row[:].unsqueeze(1).broadcast_to((1, GROUP, QL)),
                        start=True,
                        stop=False,
                        skip_group_check=True,
                    )
                    for t in range(GROUP):
                        for k in range(2):
                            nc.tensor.matmul(
                                sps[:, t, :],
                                cbT[:, 2 * t + k, :],
                                R[:, k, :],
                                start=False,
                                stop=(k == 1),
                                skip_group_check=True,
                            )
                    s_sb = ssb_pool.tile([128, GROUP, QL], F32)
                    nc.scalar.copy(s_sb[:], sps[:])
                    nc.scalar.dma_start(
                        s_d.ap()[n, lo:hi, :].rearrange("(t p) q -> p t q", p=128),
                        s_sb[:],
                    )
                    mg = m_pool.tile([128, GROUP], F32)
                    nc.vector.reduce_max(mg[:], sps[:], axis=mybir.AxisListType.X)
                    nc.scalar.activation(
                        U[:, g * GROUP : (g + 1) * GROUP],
                        mg[:],
                        mybir.ActivationFunctionType.Exp,
                    )
                    for t in range(GROUP):
                        gt = g * GROUP + t
                        nc.tensor.matmul(
                            qc_ps[:, 0:D],
                            U[:, gt : gt + 1],
                            cb[:, t, :],
                            start=(gt == 0),
                            stop=(gt == T - 1),
                            skip_group_check=True,
                        )

                # Z = sum(U) via ones^T @ U then a free-dim reduce; q2c = qc / Z
                nc.tensor.matmul(
                    qc_ps[:, D : D + T],
                    ones[:, 0:1],
                    U[:],
                    start=True,
                    stop=True,
                    skip_group_check=True,
                )
                zz = qprep.tile([1, 1], F32, tag="zz")
                nc.vector.reduce_sum(
                    zz[:], qc_ps[0:1, D : D + T], axis=mybir.AxisListType.X
                )
                rz = qprep.tile([1, 1], F32, tag="rz")
                nc.vector.reciprocal(rz[:], zz[:])
                q2c_sb = out_pool.tile([1, D], F32)
                nc.scalar.activation(
                    q2c_sb[:],
                    qc_ps[0:1, 0:D],
                    mybir.ActivationFunctionType.Copy,
                    bias=0.0,
                    scale=rz[0:1, 0:1],
                )
                nc.scalar.dma_start(q2c_d.ap()[n].unsqueeze(0), q2c_sb[:])

    nc.compile()
    return nc


_NC_CACHE = {}


def _get_nc(key):
    if key not in _NC_CACHE:
        _NC_CACHE[key] = build_kernel(*key)
    return _NC_CACHE[key]


def kernel(c, q, w_c, b_c, w_q, b_q, w_cq, b_cq, **run_kwargs):
    N, CL, D = c.shape
    _, QL, _ = q.shape
    n_per = N // N_CORES
    nc = _get_nc((n_per, CL, QL, D))

    ident = np.eye(64, dtype=np.float32)
    in_maps = []
    for i in range(N_CORES):
        sl = slice(i * n_per, (i + 1) * n_per)
        in_maps.append(
            {
                "c": np.ascontiguousarray(c[sl], dtype=np.float32),
                "q": np.ascontiguousarray(q[sl], dtype=np.float32),
                "w_c": np.asarray(w_c, dtype=np.float32),
                "w_q": np.asarray(w_q, dtype=np.float32),
                "w_cq": np.asarray(w_cq, dtype=np.float32),
                "b_c": np.asarray(b_c, dtype=np.float32),
                "b_q": np.asarray(b_q, dtype=np.float32),
                "b_cq": np.asarray(b_cq, dtype=np.float32),
                "ident": ident,
            }
        )

    res = run_bass_kernel_spmd(
        nc, in_maps, core_ids=list(range(N_CORES)), **run_kwargs
    )
    q2c = np.concatenate([res.results[i]["q2c_out"] for i in range(N_CORES)], axis=0)
    s = np.concatenate([res.results[i]["s_out"] for i in range(N_CORES)], axis=0)
    if run_kwargs:
        return (q2c, s), res
    return q2c, s


# revision 12
# speedup vs baseline: 1.1521x; 1.0556x over previous
"""BiDAF attention-flow kernel for Trainium2, data-parallel over batch on 8 cores.

Reference computation (per batch element n):
    s_c = c @ w_c + b_c                       # [CL]
    s_q = q @ w_q + b_q                       # [QL]
    cq  = (c * w_cq) @ q^T + b_cq             # [CL, QL]
    s   = cq + s_c[:, None] + s_q[None, :]    # [CL, QL]
    b_att = softmax(max(s, axis=1))           # [CL]
    q2c  = b_att @ c                          # [D]
    return q2c, s

Device mapping (per core, 2 batch elements):
  - R[d, q] = w_cq[d] * q[n, q, d] + w_c[d] folds the s_c term into the GEMM
    (sum_d c[cl,d] * w_c[d] = s_c[cl]); the s_q + b terms enter via a K=1
    ones-row matmul that pre-fills each PSUM bank.
  - c is DMA-loaded with an inline f32->bf16 cast (SWDGE), transposed on-chip
    by the DMA xbar (one [128, 2048] -> [128, 16, 128] transpose per 8-tile
    group) to give d-major lhsT chunks for the TensorE GEMM.
  - softmax skips the max-subtraction (values are O(10), exp is safe in f32)
    so q2c accumulates per-tile with no global barrier:
    q2c = (sum_cl exp(m_cl) * c[cl,:]) / sum_cl exp(m_cl).
"""

import numpy as np

import concourse.bacc as bacc
import concourse.mybir as mybir
import concourse.tile as tile
from concourse.bass_utils import run_bass_kernel_spmd

F32 = mybir.dt.float32
F32R = mybir.dt.float32r
BF16 = mybir.dt.bfloat16

N_CORES = 8
N_FULL, CL_FULL, QL_FULL, D_FULL = 16, 4096, 64, 256


def build_kernel(N_per=2, CL=4096, QL=64, D=256, GROUP=8):
    """Build the bass program for one core processing N_per batch elements."""
    assert D == 256 and QL == 64
    T = CL // 128            # cl tiles per batch elem
    G = T // GROUP           # tile groups per batch elem
    assert G * GROUP == T

    nc = bacc.Bacc("TRN2", target_bir_lowering=False, debug=False)

    c_d = nc.dram_tensor("c", [N_per, CL, D], F32, kind="ExternalInput")
    q_d = nc.dram_tensor("q", [N_per, QL, D], F32, kind="ExternalInput")
    w_c_d = nc.dram_tensor("w_c", [D], F32, kind="ExternalInput")
    w_q_d = nc.dram_tensor("w_q", [D], F32, kind="ExternalInput")
    w_cq_d = nc.dram_tensor("w_cq", [D], F32, kind="ExternalInput")
    b_c_d = nc.dram_tensor("b_c", [1], F32, kind="ExternalInput")
    b_q_d = nc.dram_tensor("b_q", [1], F32, kind="ExternalInput")
    b_cq_d = nc.dram_tensor("b_cq", [1], F32, kind="ExternalInput")
    ident_d = nc.dram_tensor("ident", [64, 64], F32, kind="ExternalInput")

    s_d = nc.dram_tensor("s_out", [N_per, CL, QL], F32, kind="ExternalOutput")
    q2c_d = nc.dram_tensor("q2c_out", [N_per, D], F32, kind="ExternalOutput")

    with tile.TileContext(nc) as tc:
        with (
            tc.tile_pool(name="const", bufs=1) as const_pool,
            tc.tile_pool(name="qprep", bufs=2) as qprep,
            tc.tile_pool(name="qpsum", bufs=1, space="PSUM") as qpsum,
            tc.tile_pool(name="cb", bufs=8) as cb_pool,
            tc.tile_pool(name="cbT", bufs=8) as cbT_pool,
            tc.tile_pool(name="spsum", bufs=4, space="PSUM") as spsum_pool,
            tc.tile_pool(name="ssb", bufs=4) as ssb_pool,
            tc.tile_pool(name="mg", bufs=3) as m_pool,
            tc.tile_pool(name="upool", bufs=2) as u_pool,
            tc.tile_pool(name="qcpsum", bufs=2, space="PSUM") as qc_pool,
            tc.tile_pool(name="outsb", bufs=2) as out_pool,
        ):
            # ---- constants ----
            ident = const_pool.tile([64, 64], F32)
            nc.scalar.dma_start(ident[:], ident_d.ap())
            ones = const_pool.tile([128, 128], BF16)
            nc.vector.memset(ones[:], 1.0)
            w_c_t = const_pool.tile([128, 2], F32, tag="w_c_t")
            w_cq_t = const_pool.tile([128, 2], F32, tag="w_cq_t")
            for w_t, w_dram in ((w_c_t, w_c_d), (w_cq_t, w_cq_d)):
                nc.scalar.dma_start(w_t[:], w_dram.ap().rearrange("(k p) -> p k", p=128))
            w_q_b = const_pool.tile([128, 2], BF16, tag="w_q_b")
            nc.gpsimd.dma_start(w_q_b[:], w_q_d.ap().rearrange("(k p) -> p k", p=128))
            b3 = const_pool.tile([1, 3], F32, tag="b3")
            for i, b_dram in enumerate((b_c_d, b_q_d, b_cq_d)):
                nc.scalar.dma_start(b3[:, i : i + 1], b_dram.ap().unsqueeze(0))
            bsum = const_pool.tile([1, 1], F32, tag="bsum")
            nc.vector.reduce_sum(bsum[:], b3[:], axis=mybir.AxisListType.X)

            # ---- q-side prep for all batch elems first ----
            Rs, sq_rows, Us, qc_pss = [], [], [], []
            for n in range(N_per):
                q_sb = qprep.tile([QL, D], F32, tag="q_sb")
                nc.scalar.dma_start(q_sb[:], q_d.ap()[n])
                qt_ps = qpsum.tile([128, 2, QL], F32, tag="qt_ps")
                for k in range(2):
                    nc.tensor.transpose(
                        qt_ps[:, k, :], q_sb[:, 128 * k : 128 * (k + 1)], ident[:]
                    )
                qt_b16 = qprep.tile([128, 2, QL], BF16, tag="qt_b16")
                R = qprep.tile([128, 2, QL], BF16, tag="R")
                for k in range(2):
                    nc.scalar.copy(qt_b16[:, k, :], qt_ps[:, k, :])
                    nc.scalar.activation(
                        R[:, k, :],
                        qt_ps[:, k, :],
                        mybir.ActivationFunctionType.Identity,
                        bias=w_c_t[:, k : k + 1],
                        scale=w_cq_t[:, k : k + 1],
                    )
                qc_ps = qc_pool.tile([1, 512], F32)
                for k in range(2):
                    nc.tensor.matmul(
                        qc_ps[:, 320 : 320 + QL],
                        w_q_b[:, k : k + 1],
                        qt_b16[:, k, :],
                        start=(k == 0),
                        stop=(k == 1),
                        skip_group_check=True,
                    )
                sq_row = qprep.tile([1, QL], BF16, tag="sq_row")
                nc.scalar.activation(
                    sq_row[:],
                    qc_ps[0:1, 320 : 320 + QL],
                    mybir.ActivationFunctionType.Identity,
                    bias=bsum[:, 0:1],
                )
                U = u_pool.tile([128, T], BF16)
                Rs.append(R); sq_rows.append(sq_row); Us.append(U); qc_pss.append(qc_ps)

            # ---- main loop: interleave the two batch elems group by group ----
            for gi in range(N_per * G):
                n, g = gi % N_per, gi // N_per
                R, sq_row, U, qc_ps = Rs[n], sq_rows[n], Us[n], qc_pss[n]
                lo = g * GROUP * 128
                hi = (g + 1) * GROUP * 128
                # load 8 cl-tiles, casting f32 -> bf16 inline (SWDGE)
                cb = cb_pool.tile([128, GROUP, D], BF16)
                nc.gpsimd.dma_start(
                    cb[:],
                    c_d.ap()[n, lo:hi, :].rearrange("(t p) d -> p t d", p=128),
                )
                # xbar transpose: [128cl, (t,d)] -> chunk-major d-on-partition
                cbT = cbT_pool.tile([128, 2 * GROUP, 128], BF16)
                nc.sync.dma_start_transpose(
                    cbT[:], cb[:].rearrange("p t d -> p (t d)")
                )
                # s = ones^T @ (sq_row + bsum broadcast)  +  c @ R
                sps = spsum_pool.tile([128, GROUP, QL], F32)
                nc.tensor.matmul(
                    sps[:],
                    ones[0:1, :],
                    sq_row[:].unsqueeze(1).broadcast_to((1, GROUP, QL)),
                    start=True,
                    stop=False,
                    skip_group_check=True,
                )
                for t in range(GROUP):
                    for k in range(2):
                        nc.tensor.matmul(
                            sps[:, t, :],
                            cbT[:, 2 * t + k, :],
                            R[:, k, :],
                            start=False,
                            stop=(k == 1),
                            skip_group_check=True,
                        )
                # softmax branch first (it recycles cb via the q2c matmuls)
                mg = m_pool.tile([128, GROUP], F32)
                nc.vector.reduce_max(mg[:], sps[:], axis=mybir.AxisListType.X)
                nc.scalar.activation(
                    U[:, g * GROUP : (g + 1) * GROUP],
                    mg[:],
                    mybir.ActivationFunctionType.Exp,
                )
                for t in range(GROUP):
                    gt = g * GROUP + t
                    nc.tensor.matmul(
                        qc_ps[:, 0:D],
                        U[:, gt : gt + 1],
                        cb[:, t, :],
                        start=(gt == 0),
                        stop=(gt == T - 1),
                        skip_group_check=True,
                    )
                # s epilogue + store
                s_sb = ssb_pool.tile([128, GROUP, QL], F32)
                nc.scalar.copy(s_sb[:], sps[:])
                nc.scalar.dma_start(
                    s_d.ap()[n, lo:hi, :].rearrange("(t p) q -> p t q", p=128),
                    s_sb[:],
                )

            for n in range(N_per):
                U, qc_ps = Us[n], qc_pss[n]
                # Z = sum(U) via ones^T @ U then a free-dim reduce; q2c = qc / Z
                nc.tensor.matmul(
                    qc_ps[:, D : D + T],
                    ones[:, 0:1],
                    U[:],
                    start=True,
                    stop=True,
                    skip_group_check=True,
                )
                zz = qprep.tile([1, 1], F32, tag="zz")
                nc.vector.reduce_sum(
                    zz[:], qc_ps[0:1, D : D + T], axis=mybir.AxisListType.X
                )
                rz = qprep.tile([1, 1], F32, tag="rz")
                nc.vector.reciprocal(rz[:], zz[:])
                q2c_sb = out_pool.tile([1, D], F32)
                nc.scalar.activation(
                    q2c_sb[:],
                    qc_ps[0:1, 0:D],
                    mybir.ActivationFunctionType.Copy,
                    bias=0.0,
                    scale=rz[0:1, 0:1],
                )
                nc.scalar.dma_start(q2c_d.ap()[n].unsqueeze(0), q2c_sb[:])

    nc.compile()
    return nc


_NC_CACHE = {}


def _get_nc(key):
    if key not in _NC_CACHE:
        _NC_CACHE[key] = build_kernel(*key)
    return _NC_CACHE[key]


def kernel(c, q, w_c, b_c, w_q, b_q, w_cq, b_cq, **run_kwargs):
    N, CL, D = c.shape
    _, QL, _ = q.shape
    n_per = N // N_CORES
    nc = _get_nc((n_per, CL, QL, D))

    ident = np.eye(64, dtype=np.float32)
    in_maps = []
    for i in range(N_CORES):
        sl = slice(i * n_per, (i + 1) * n_per)
        in_maps.append(
            {
                "c": np.ascontiguousarray(c[sl], dtype=np.float32),
                "q": np.ascontiguousarray(q[sl], dtype=np.float32),
                "w_c": np.asarray(w_c, dtype=np.float32),
                "w_q": np.asarray(w_q, dtype=np.float32),
                "w_cq": np.asarray(w_cq, dtype=np.float32),
                "b_c": np.asarray(b_c, dtype=np.float32),
                "b_q": np.asarray(b_q, dtype=np.float32),
                "b_cq": np.asarray(b_cq, dtype=np.float32),
                "ident": ident,
            }
        )

    res = run_bass_kernel_spmd(
        nc, in_maps, core_ids=list(range(N_CORES)), **run_kwargs
    )
    q2c = np.concatenate([res.results[i]["q2c_out"] for i in range(N_CORES)], axis=0)
    s = np.concatenate([res.results[i]["s_out"] for i in range(N_CORES)], axis=0)
    if run_kwargs:
        return (q2c, s), res
    return q2c, s


# revision 13
# speedup vs baseline: 1.1690x; 1.0146x over previous
"""BiDAF attention-flow kernel for Trainium2, data-parallel over batch on 8 cores.

Reference computation (per batch element n):
    s_c = c @ w_c + b_c                       # [CL]
    s_q = q @ w_q + b_q                       # [QL]
    cq  = (c * w_cq) @ q^T + b_cq             # [CL, QL]
    s   = cq + s_c[:, None] + s_q[None, :]    # [CL, QL]
    b_att = softmax(max(s, axis=1))           # [CL]
    q2c  = b_att @ c                          # [D]
    return q2c, s

Device mapping (per core, 2 batch elements):
  - R[d, q] = w_cq[d] * q[n, q, d] + w_c[d] folds the s_c term into the GEMM
    (sum_d c[cl,d] * w_c[d] = s_c[cl]); the s_q + b terms enter via a K=1
    ones-row matmul that pre-fills each PSUM bank.
  - c is DMA-loaded with an inline f32->bf16 cast (SWDGE), transposed on-chip
    by the DMA xbar (one [128, 2048] -> [128, 16, 128] transpose per 8-tile
    group) to give d-major lhsT chunks for the TensorE GEMM.
  - softmax skips the max-subtraction (values are O(10), exp is safe in f32)
    so q2c accumulates per-tile with no global barrier:
    q2c = (sum_cl exp(m_cl) * c[cl,:]) / sum_cl exp(m_cl).
"""

import numpy as np

import concourse.bacc as bacc
import concourse.mybir as mybir
import concourse.tile as tile
from concourse.bass_utils import run_bass_kernel_spmd

F32 = mybir.dt.float32
F32R = mybir.dt.float32r
BF16 = mybir.dt.bfloat16

N_CORES = 8
N_FULL, CL_FULL, QL_FULL, D_FULL = 16, 4096, 64, 256


def build_kernel(N_per=2, CL=4096, QL=64, D=256, GROUP=8):
    """Build the bass program for one core processing N_per batch elements."""
    assert D == 256 and QL == 64
    T = CL // 128            # cl tiles per batch elem
    G = T // GROUP           # tile groups per batch elem
    assert G * GROUP == T

    nc = bacc.Bacc(
        "TRN2",
        target_bir_lowering=False,
        debug=False,
        dynamic_dma_scratch_size=65536,
    )

    c_d = nc.dram_tensor("c", [N_per, CL, D], F32, kind="ExternalInput")
    q_d = nc.dram_tensor("q", [N_per, QL, D], F32, kind="ExternalInput")
    w_c_d = nc.dram_tensor("w_c", [D], F32, kind="ExternalInput")
    w_q_d = nc.dram_tensor("w_q", [D], F32, kind="ExternalInput")
    w_cq_d = nc.dram_tensor("w_cq", [D], F32, kind="ExternalInput")
    b_c_d = nc.dram_tensor("b_c", [1], F32, kind="ExternalInput")
    b_q_d = nc.dram_tensor("b_q", [1], F32, kind="ExternalInput")
    b_cq_d = nc.dram_tensor("b_cq", [1], F32, kind="ExternalInput")
    ident_d = nc.dram_tensor("ident", [64, 64], F32, kind="ExternalInput")

    s_d = nc.dram_tensor("s_out", [N_per, CL, QL], F32, kind="ExternalOutput")
    q2c_d = nc.dram_tensor("q2c_out", [N_per, D], F32, kind="ExternalOutput")

    with tile.TileContext(nc) as tc:
        with (
            tc.tile_pool(name="const", bufs=1) as const_pool,
            tc.tile_pool(name="qprep", bufs=2) as qprep,
            tc.tile_pool(name="qpsum", bufs=1, space="PSUM") as qpsum,
            tc.tile_pool(name="cb", bufs=8) as cb_pool,
            tc.tile_pool(name="cbT", bufs=8) as cbT_pool,
            tc.tile_pool(name="spsum", bufs=4, space="PSUM") as spsum_pool,
            tc.tile_pool(name="ssb", bufs=4) as ssb_pool,
            tc.tile_pool(name="mg", bufs=3) as m_pool,
            tc.tile_pool(name="upool", bufs=2) as u_pool,
            tc.tile_pool(name="qcpsum", bufs=2, space="PSUM") as qc_pool,
            tc.tile_pool(name="outsb", bufs=2) as out_pool,
        ):
            # ---- constants ----
            ident = const_pool.tile([64, 64], F32)
            nc.scalar.dma_start(ident[:], ident_d.ap())
            ones = const_pool.tile([128, 128], BF16)
            nc.vector.memset(ones[:], 1.0)
            w_c_t = const_pool.tile([128, 2], F32, tag="w_c_t")
            w_cq_t = const_pool.tile([128, 2], F32, tag="w_cq_t")
            for w_t, w_dram in ((w_c_t, w_c_d), (w_cq_t, w_cq_d)):
                nc.scalar.dma_start(w_t[:], w_dram.ap().rearrange("(k p) -> p k", p=128))
            w_q_b = const_pool.tile([128, 2], BF16, tag="w_q_b")
            nc.gpsimd.dma_start(w_q_b[:], w_q_d.ap().rearrange("(k p) -> p k", p=128))
            b3 = const_pool.tile([1, 3], F32, tag="b3")
            for i, b_dram in enumerate((b_c_d, b_q_d, b_cq_d)):
                nc.scalar.dma_start(b3[:, i : i + 1], b_dram.ap().unsqueeze(0))
            bsum = const_pool.tile([1, 1], F32, tag="bsum")
            nc.vector.reduce_sum(bsum[:], b3[:], axis=mybir.AxisListType.X)

            # ---- q-side prep for all batch elems first ----
            Rs, sq_rows, Us, qc_pss = [], [], [], []
            for n in range(N_per):
                q_sb = qprep.tile([QL, D], F32, tag="q_sb")
                nc.scalar.dma_start(q_sb[:], q_d.ap()[n])
                qt_ps = qpsum.tile([128, 2, QL], F32, tag="qt_ps")
                for k in range(2):
                    nc.tensor.transpose(
                        qt_ps[:, k, :], q_sb[:, 128 * k : 128 * (k + 1)], ident[:]
                    )
                qt_b16 = qprep.tile([128, 2, QL], BF16, tag="qt_b16")
                R = qprep.tile([128, 2, QL], BF16, tag="R")
                for k in range(2):
                    nc.scalar.copy(qt_b16[:, k, :], qt_ps[:, k, :])
                    nc.scalar.activation(
                        R[:, k, :],
                        qt_ps[:, k, :],
                        mybir.ActivationFunctionType.Identity,
                        bias=w_c_t[:, k : k + 1],
                        scale=w_cq_t[:, k : k + 1],
                    )
                qc_ps = qc_pool.tile([1, 512], F32)
                for k in range(2):
                    nc.tensor.matmul(
                        qc_ps[:, 320 : 320 + QL],
                        w_q_b[:, k : k + 1],
                        qt_b16[:, k, :],
                        start=(k == 0),
                        stop=(k == 1),
                        skip_group_check=True,
                    )
                sq_row = qprep.tile([1, QL], BF16, tag="sq_row")
                nc.scalar.activation(
                    sq_row[:],
                    qc_ps[0:1, 320 : 320 + QL],
                    mybir.ActivationFunctionType.Identity,
                    bias=bsum[:, 0:1],
                )
                U = u_pool.tile([128, T], BF16)
                Rs.append(R); sq_rows.append(sq_row); Us.append(U); qc_pss.append(qc_ps)

            # ---- main loop: interleave the two batch elems group by group ----
            for gi in range(N_per * G):
                n, g = gi % N_per, gi // N_per
                R, sq_row, U, qc_ps = Rs[n], sq_rows[n], Us[n], qc_pss[n]
                lo = g * GROUP * 128
                hi = (g + 1) * GROUP * 128
                # load 8 cl-tiles, casting f32 -> bf16 inline (SWDGE)
                cb = cb_pool.tile([128, GROUP, D], BF16)
                nc.gpsimd.dma_start(
                    cb[:],
                    c_d.ap()[n, lo:hi, :].rearrange("(t p) d -> p t d", p=128),
                )
                # xbar transpose: [128cl, (t,d)] -> chunk-major d-on-partition
                cbT = cbT_pool.tile([128, 2 * GROUP, 128], BF16)
                nc.sync.dma_start_transpose(
                    cbT[:], cb[:].rearrange("p t d -> p (t d)")
                )
                # s = ones^T @ (sq_row + bsum broadcast)  +  c @ R
                sps = spsum_pool.tile([128, GROUP, QL], F32)
                nc.tensor.matmul(
                    sps[:],
                    ones[0:1, :],
                    sq_row[:].unsqueeze(1).broadcast_to((1, GROUP, QL)),
                    start=True,
                    stop=False,
                    skip_group_check=True,
                )
                for t in range(GROUP):
                    for k in range(2):
                        nc.tensor.matmul(
                            sps[:, t, :],
                            cbT[:, 2 * t + k, :],
                            R[:, k, :],
                            start=False,
                            stop=(k == 1),
                            skip_group_check=True,
                        )
                # softmax branch first (it recycles cb via the q2c matmuls)
                mg = m_pool.tile([128, GROUP], F32)
                nc.vector.reduce_max(mg[:], sps[:], axis=mybir.AxisListType.X)
                nc.scalar.activation(
                    U[:, g * GROUP : (g + 1) * GROUP],
                    mg[:],
                    mybir.ActivationFunctionType.Exp,
                )
                for t in range(GROUP):
                    gt = g * GROUP + t
                    nc.tensor.matmul(
                        qc_ps[:, 0:D],
                        U[:, gt : gt + 1],
                        cb[:, t, :],
                        start=(gt == 0),
                        stop=(gt == T - 1),
                        skip_group_check=True,
                    )
                # s epilogue + store
                s_sb = ssb_pool.tile([128, GROUP, QL], F32)
                nc.scalar.copy(s_sb[:], sps[:])
                nc.scalar.dma_start(
                    s_d.ap()[n, lo:hi, :].rearrange("(t p) q -> p t q", p=128),
                    s_sb[:],
                )

            for n in range(N_per):
                U, qc_ps = Us[n], qc_pss[n]
                # Z = sum(U) via ones^T @ U then a free-dim reduce; q2c = qc / Z
                nc.tensor.matmul(
                    qc_ps[:, D : D + T],
                    ones[:, 0:1],
                    U[:],
                    start=True,
                    stop=True,
                    skip_group_check=True,
                )
                zz = qprep.tile([1, 1], F32, tag="zz")
                nc.vector.reduce_sum(
                    zz[:], qc_ps[0:1, D : D + T], axis=mybir.AxisListType.X
                )
                rz = qprep.tile([1, 1], F32, tag="rz")
                nc.vector.reciprocal(rz[:], zz[:])
                q2c_sb = out_pool.tile([1, D], F32)
                nc.scalar.activation(
                    q2c_sb[:],
                    qc_ps[0:1, 0:D],
                    mybir.ActivationFunctionType.Copy,
                    bias=0.0,
                    scale=rz[0:1, 0:1],
                )
                nc.scalar.dma_start(q2c_d.ap()[n].unsqueeze(0), q2c_sb[:])

    nc.compile()
    return nc


_NC_CACHE = {}


def _get_nc(key):
    if key not in _NC_CACHE:
        _NC_CACHE[key] = build_kernel(*key)
    return _NC_CACHE[key]


def kernel(c, q, w_c, b_c, w_q, b_q, w_cq, b_cq, **run_kwargs):
    N, CL, D = c.shape
    _, QL, _ = q.shape
    n_per = N // N_CORES
    nc = _get_nc((n_per, CL, QL, D))

    ident = np.eye(64, dtype=np.float32)
    in_maps = []
    for i in range(N_CORES):
        sl = slice(i * n_per, (i + 1) * n_per)
        in_maps.append(
            {
                "c": np.ascontiguousarray(c[sl], dtype=np.float32),
                "q": np.ascontiguousarray(q[sl], dtype=np.float32),
                "w_c": np.asarray(w_c, dtype=np.float32),
                "w_q": np.asarray(w_q, dtype=np.float32),
                "w_cq": np.asarray(w_cq, dtype=np.float32),
                "b_c": np.asarray(b_c, dtype=np.float32),
                "b_q": np.asarray(b_q, dtype=np.float32),
                "b_cq": np.asarray(b_cq, dtype=np.float32),
                "ident": ident,
            }
        )

    res = run_bass_kernel_spmd(
        nc, in_maps, core_ids=list(range(N_CORES)), **run_kwargs
    )
    q2c = np.concatenate([res.results[i]["q2c_out"] for i in range(N_CORES)], axis=0)
    s = np.concatenate([res.results[i]["s_out"] for i in range(N_CORES)], axis=0)
    if run_kwargs:
        return (q2c, s), res
    return q2c, s


# revision 15
# speedup vs baseline: 1.9871x; 1.6998x over previous
"""BiDAF attention-flow kernel for Trainium2, data-parallel over batch on 8 cores.

Reference computation (per batch element n):
    s_c = c @ w_c + b_c                       # [CL]
    s_q = q @ w_q + b_q                       # [QL]
    cq  = (c * w_cq) @ q^T + b_cq             # [CL, QL]
    s   = cq + s_c[:, None] + s_q[None, :]    # [CL, QL]
    b_att = softmax(max(s, axis=1))           # [CL]
    q2c  = b_att @ c                          # [D]
    return q2c, s

Device mapping (per core, 2 batch elements):
  - R[d, q] = w_cq[d] * q[n, q, d] + w_c[d] folds the s_c term into the GEMM
    (sum_d c[cl,d] * w_c[d] = s_c[cl]); the s_q + b terms enter via a K=1
    ones-row matmul that pre-fills each PSUM bank.
  - c is DMA-loaded with an inline f32->bf16 cast (SWDGE), transposed on-chip
    by the DMA xbar (one [128, 2048] -> [128, 16, 128] transpose per 8-tile
    group) to give d-major lhsT chunks for the TensorE GEMM.
  - softmax skips the max-subtraction (values are O(10), exp is safe in f32)
    so q2c accumulates per-tile with no global barrier:
    q2c = (sum_cl exp(m_cl) * c[cl,:]) / sum_cl exp(m_cl).
"""

import numpy as np

import concourse.bacc as bacc
import concourse.mybir as mybir
import concourse.tile as tile
from concourse.bass_utils import run_bass_kernel_spmd

F32 = mybir.dt.float32
F32R = mybir.dt.float32r
BF16 = mybir.dt.bfloat16

N_CORES = 8
N_FULL, CL_FULL, QL_FULL, D_FULL = 16, 4096, 64, 256


def build_kernel(N_per=2, CL=4096, QL=64, D=256, GROUP=8):
    """Build the bass program for one core processing N_per batch elements."""
    assert D == 256 and QL == 64
    T = CL // 128            # cl tiles per batch elem
    G = T // GROUP           # tile groups per batch elem
    assert G * GROUP == T

    nc = bacc.Bacc(
        "TRN2",
        target_bir_lowering=False,
        debug=False,
        dynamic_dma_scratch_size=65536,
    )

    c_d = nc.dram_tensor("c", [N_per, CL, D], F32, kind="ExternalInput")
    q_d = nc.dram_tensor("q", [N_per, QL, D], F32, kind="ExternalInput")
    w_c_d = nc.dram_tensor("w_c", [D], F32, kind="ExternalInput")
    w_q_d = nc.dram_tensor("w_q", [D], F32, kind="ExternalInput")
    w_cq_d = nc.dram_tensor("w_cq", [D], F32, kind="ExternalInput")
    b_c_d = nc.dram_tensor("b_c", [1], F32, kind="ExternalInput")
    b_q_d = nc.dram_tensor("b_q", [1], F32, kind="ExternalInput")
    b_cq_d = nc.dram_tensor("b_cq", [1], F32, kind="ExternalInput")
    ident_d = nc.dram_tensor("ident", [64, 64], F32, kind="ExternalInput")
    identb_d = nc.dram_tensor("identb", [128, 128], BF16, kind="ExternalInput")

    s_d = nc.dram_tensor("s_out", [N_per, CL, QL], F32, kind="ExternalOutput")
    q2c_d = nc.dram_tensor("q2c_out", [N_per, D], F32, kind="ExternalOutput")

    with tile.TileContext(nc) as tc:
        with (
            tc.tile_pool(name="const", bufs=1) as const_pool,
            tc.tile_pool(name="qprep", bufs=2) as qprep,
                        tc.tile_pool(name="cb", bufs=8) as cb_pool,
            tc.tile_pool(name="cbT", bufs=4) as cbT_pool,
            tc.tile_pool(name="spsum", bufs=3, space="PSUM") as spsum_pool,
            tc.tile_pool(name="tpsum", bufs=3, space="PSUM") as tpsum_pool,
            tc.tile_pool(name="ssb", bufs=4) as ssb_pool,
            tc.tile_pool(name="mg", bufs=3) as m_pool,
            tc.tile_pool(name="upool", bufs=2) as u_pool,
            tc.tile_pool(name="qcpsum", bufs=2, space="PSUM") as qc_pool,
            tc.tile_pool(name="outsb", bufs=2) as out_pool,
        ):
            # ---- constants ----
            ident = const_pool.tile([64, 64], F32)
            nc.scalar.dma_start(ident[:], ident_d.ap())
            identb = const_pool.tile([128, 128], BF16, tag="identb")
            nc.scalar.dma_start(identb[:], identb_d.ap())
            ones = const_pool.tile([128, 128], BF16)
            nc.vector.memset(ones[:], 1.0)
            w_c_t = const_pool.tile([128, 2], F32, tag="w_c_t")
            w_cq_t = const_pool.tile([128, 2], F32, tag="w_cq_t")
            for w_t, w_dram in ((w_c_t, w_c_d), (w_cq_t, w_cq_d)):
                nc.scalar.dma_start(w_t[:], w_dram.ap().rearrange("(k p) -> p k", p=128))
            w_q_b = const_pool.tile([128, 2], BF16, tag="w_q_b")
            nc.gpsimd.dma_start(w_q_b[:], w_q_d.ap().rearrange("(k p) -> p k", p=128))
            b3 = const_pool.tile([1, 3], F32, tag="b3")
            for i, b_dram in enumerate((b_c_d, b_q_d, b_cq_d)):
                nc.scalar.dma_start(b3[:, i : i + 1], b_dram.ap().unsqueeze(0))
            bsum = const_pool.tile([1, 1], F32, tag="bsum")
            nc.vector.reduce_sum(bsum[:], b3[:], axis=mybir.AxisListType.X)

            # ---- q-side prep for all batch elems first ----
            Rs, sq_rows, Us, qc_pss = [], [], [], []
            for n in range(N_per):
                q_sb = qprep.tile([QL, D], F32, tag="q_sb")
                nc.scalar.dma_start(q_sb[:], q_d.ap()[n])
                qt_ps = tpsum_pool.tile([128, 2, QL], F32, tag="tp")
                for k in range(2):
                    nc.tensor.transpose(
                        qt_ps[:, k, :], q_sb[:, 128 * k : 128 * (k + 1)], ident[:]
                    )
                qt_b16 = qprep.tile([128, 2, QL], BF16, tag="qt_b16")
                R = qprep.tile([128, 2, QL], BF16, tag="R")
                for k in range(2):
                    nc.scalar.copy(qt_b16[:, k, :], qt_ps[:, k, :])
                    nc.scalar.activation(
                        R[:, k, :],
                        qt_ps[:, k, :],
                        mybir.ActivationFunctionType.Identity,
                        bias=w_c_t[:, k : k + 1],
                        scale=w_cq_t[:, k : k + 1],
                    )
                qc_ps = qc_pool.tile([1, 512], F32)
                for k in range(2):
                    nc.tensor.matmul(
                        qc_ps[:, 320 : 320 + QL],
                        w_q_b[:, k : k + 1],
                        qt_b16[:, k, :],
                        start=(k == 0),
                        stop=(k == 1),
                        skip_group_check=True,
                    )
                sq_row = qprep.tile([1, QL], BF16, tag="sq_row")
                nc.scalar.activation(
                    sq_row[:],
                    qc_ps[0:1, 320 : 320 + QL],
                    mybir.ActivationFunctionType.Identity,
                    bias=bsum[:, 0:1],
                )
                U = u_pool.tile([128, T], BF16)
                Rs.append(R); sq_rows.append(sq_row); Us.append(U); qc_pss.append(qc_ps)

            # ---- main loop: interleave the two batch elems group by group ----
            for gi in range(N_per * G):
                n, g = gi % N_per, gi // N_per
                R, sq_row, U, qc_ps = Rs[n], sq_rows[n], Us[n], qc_pss[n]
                lo = g * GROUP * 128
                hi = (g + 1) * GROUP * 128
                # load 8 cl-tiles, casting f32 -> bf16 inline (SWDGE)
                cb = cb_pool.tile([128, GROUP, D], BF16)
                nc.gpsimd.dma_start(
                    cb[:],
                    c_d.ap()[n, lo:hi, :].rearrange("(t p) d -> p t d", p=128),
                )
                # PE transpose: cb [128cl, (t,d)] -> cbT chunks [128d, 128cl]
                cbT = cbT_pool.tile([128, 2 * GROUP, 128], BF16)
                for cc in range(2):
                    tps = tpsum_pool.tile([128, GROUP, 128], BF16, tag="tp")
                    for j in range(GROUP):
                        jj = cc * GROUP + j      # chunk index 0..15
                        t, k = jj // 2, jj % 2
                        nc.tensor.transpose(
                            tps[:, j, :],
                            cb[:, t, 128 * k : 128 * (k + 1)],
                            identb[:],
                        )
                    if cc == 0:
                        nc.scalar.copy(
                            cbT[:, cc * GROUP : (cc + 1) * GROUP, :], tps[:]
                        )
                    else:
                        nc.vector.tensor_copy(
                            cbT[:, cc * GROUP : (cc + 1) * GROUP, :], tps[:]
                        )
                # s = ones^T @ (sq_row + bsum broadcast)  +  c @ R
                sps = spsum_pool.tile([128, GROUP, QL], F32)
                nc.tensor.matmul(
                    sps[:],
                    ones[0:1, :],
                    sq_row[:].unsqueeze(1).broadcast_to((1, GROUP, QL)),
                    start=True,
                    stop=False,
                    skip_group_check=True,
                )
                for t in range(GROUP):
                    for k in range(2):
                        nc.tensor.matmul(
                            sps[:, t, :],
                            cbT[:, 2 * t + k, :],
                            R[:, k, :],
                            start=False,
                            stop=(k == 1),
                            skip_group_check=True,
                        )
                # softmax branch first (it recycles cb via the q2c matmuls)
                mg = m_pool.tile([128, GROUP], F32)
                nc.vector.reduce_max(mg[:], sps[:], axis=mybir.AxisListType.X)
                nc.scalar.activation(
                    U[:, g * GROUP : (g + 1) * GROUP],
                    mg[:],
                    mybir.ActivationFunctionType.Exp,
                )
                for t in range(GROUP):
                    gt = g * GROUP + t
                    nc.tensor.matmul(
                        qc_ps[:, 0:D],
                        U[:, gt : gt + 1],
                        cb[:, t, :],
                        start=(gt == 0),
                        stop=(gt == T - 1),
                        skip_group_check=True,
                    )
                # s epilogue + store
                s_sb = ssb_pool.tile([128, GROUP, QL], F32)
                nc.scalar.copy(s_sb[:], sps[:])
                nc.sync.dma_start(
                    s_d.ap()[n, lo:hi, :].rearrange("(t p) q -> p t q", p=128),
                    s_sb[:],
                )

            for n in range(N_per):
                U, qc_ps = Us[n], qc_pss[n]
                # Z = sum(U) via ones^T @ U then a free-dim reduce; q2c = qc / Z
                nc.tensor.matmul(
                    qc_ps[:, D : D + T],
                    ones[:, 0:1],
                    U[:],
                    start=True,
                    stop=True,
                    skip_group_check=True,
                )
                zz = qprep.tile([1, 1], F32, tag="zz")
                nc.vector.reduce_sum(
                    zz[:], qc_ps[0:1, D : D + T], axis=mybir.AxisListType.X
                )
                rz = qprep.tile([1, 1], F32, tag="rz")
                nc.vector.reciprocal(rz[:], zz[:])
                q2c_sb = out_pool.tile([1, D], F32)
                nc.scalar.activation(
                    q2c_sb[:],
                    qc_ps[0:1, 0:D],
                    mybir.ActivationFunctionType.Copy,
                    bias=0.0,
                    scale=rz[0:1, 0:1],
                )
                nc.scalar.dma_start(q2c_d.ap()[n].unsqueeze(0), q2c_sb[:])

    nc.compile()
    return nc


_NC_CACHE = {}


def _get_nc(key):
    if key not in _NC_CACHE:
        _NC_CACHE[key] = build_kernel(*key)
    return _NC_CACHE[key]


def kernel(c, q, w_c, b_c, w_q, b_q, w_cq, b_cq, **run_kwargs):
    N, CL, D = c.shape
    _, QL, _ = q.shape
    n_per = N // N_CORES
    nc = _get_nc((n_per, CL, QL, D))

    import ml_dtypes

    ident = np.eye(64, dtype=np.float32)
    identb = np.eye(128, dtype=np.float32).astype(ml_dtypes.bfloat16)
    in_maps = []
    for i in range(N_CORES):
        sl = slice(i * n_per, (i + 1) * n_per)
        in_maps.append(
            {
                "c": np.ascontiguousarray(c[sl], dtype=np.float32),
                "q": np.ascontiguousarray(q[sl], dtype=np.float32),
                "w_c": np.asarray(w_c, dtype=np.float32),
                "w_q": np.asarray(w_q, dtype=np.float32),
                "w_cq": np.asarray(w_cq, dtype=np.float32),
                "b_c": np.asarray(b_c, dtype=np.float32),
                "b_q": np.asarray(b_q, dtype=np.float32),
                "b_cq": np.asarray(b_cq, dtype=np.float32),
                "ident": ident,
                "identb": identb,
            }
        )

    res = run_bass_kernel_spmd(
        nc, in_maps, core_ids=list(range(N_CORES)), **run_kwargs
    )
    q2c = np.concatenate([res.results[i]["q2c_out"] for i in range(N_CORES)], axis=0)
    s = np.concatenate([res.results[i]["s_out"] for i in range(N_CORES)], axis=0)
    if run_kwargs:
        return (q2c, s), res
    return q2c, s


# revision 17
# speedup vs baseline: 2.0751x; 1.0443x over previous
"""BiDAF attention-flow kernel for Trainium2, data-parallel over batch on 8 cores.

Reference computation (per batch element n):
    s_c = c @ w_c + b_c                       # [CL]
    s_q = q @ w_q + b_q                       # [QL]
    cq  = (c * w_cq) @ q^T + b_cq             # [CL, QL]
    s   = cq + s_c[:, None] + s_q[None, :]    # [CL, QL]
    b_att = softmax(max(s, axis=1))           # [CL]
    q2c  = b_att @ c                          # [D]
    return q2c, s

Device mapping (per core, 2 batch elements):
  - R[d, q] = w_cq[d] * q[n, q, d] + w_c[d] folds the s_c term into the GEMM
    (sum_d c[cl,d] * w_c[d] = s_c[cl]); the s_q + b terms enter via a K=1
    ones-row matmul that pre-fills each PSUM bank.
  - c is DMA-loaded with an inline f32->bf16 cast (SWDGE), transposed on-chip
    by the DMA xbar (one [128, 2048] -> [128, 16, 128] transpose per 8-tile
    group) to give d-major lhsT chunks for the TensorE GEMM.
  - softmax skips the max-subtraction (values are O(10), exp is safe in f32)
    so q2c accumulates per-tile with no global barrier:
    q2c = (sum_cl exp(m_cl) * c[cl,:]) / sum_cl exp(m_cl).
"""

import numpy as np

import concourse.bacc as bacc
import concourse.mybir as mybir
import concourse.tile as tile
from concourse.bass_utils import run_bass_kernel_spmd

F32 = mybir.dt.float32
F32R = mybir.dt.float32r
BF16 = mybir.dt.bfloat16

N_CORES = 8
N_FULL, CL_FULL, QL_FULL, D_FULL = 16, 4096, 64, 256


def build_kernel(N_per=2, CL=4096, QL=64, D=256, GROUP=8):
    """Build the bass program for one core processing N_per batch elements."""
    assert D == 256 and QL == 64
    T = CL // 128            # cl tiles per batch elem
    G = T // GROUP           # tile groups per batch elem
    assert G * GROUP == T

    nc = bacc.Bacc(
        "TRN2",
        target_bir_lowering=False,
        debug=False,
        dynamic_dma_scratch_size=65536,
    )

    c_d = nc.dram_tensor("c", [N_per, CL, D], F32, kind="ExternalInput")
    q_d = nc.dram_tensor("q", [N_per, QL, D], F32, kind="ExternalInput")
    w_c_d = nc.dram_tensor("w_c", [D], F32, kind="ExternalInput")
    w_q_d = nc.dram_tensor("w_q", [D], F32, kind="ExternalInput")
    w_cq_d = nc.dram_tensor("w_cq", [D], F32, kind="ExternalInput")
    b_c_d = nc.dram_tensor("b_c", [1], F32, kind="ExternalInput")
    b_q_d = nc.dram_tensor("b_q", [1], F32, kind="ExternalInput")
    b_cq_d = nc.dram_tensor("b_cq", [1], F32, kind="ExternalInput")
    ident_d = nc.dram_tensor("ident", [64, 64], F32, kind="ExternalInput")
    identb_d = nc.dram_tensor("identb", [128, 128], BF16, kind="ExternalInput")

    s_d = nc.dram_tensor("s_out", [N_per, CL, QL], F32, kind="ExternalOutput")
    q2c_d = nc.dram_tensor("q2c_out", [N_per, D], F32, kind="ExternalOutput")

    with tile.TileContext(nc) as tc:
        with (
            tc.tile_pool(name="const", bufs=1) as const_pool,
            tc.tile_pool(name="qprep", bufs=2) as qprep,
                        tc.tile_pool(name="cb", bufs=8) as cb_pool,
            tc.tile_pool(name="cbT", bufs=4) as cbT_pool,
            tc.tile_pool(name="spsum", bufs=3, space="PSUM") as spsum_pool,
            tc.tile_pool(name="tpsum", bufs=3, space="PSUM") as tpsum_pool,
            tc.tile_pool(name="ssb", bufs=4) as ssb_pool,
            tc.tile_pool(name="mg", bufs=3) as m_pool,
            tc.tile_pool(name="upool", bufs=2) as u_pool,
            tc.tile_pool(name="qcpsum", bufs=2, space="PSUM") as qc_pool,
            tc.tile_pool(name="outsb", bufs=2) as out_pool,
        ):
            # ---- constants ----
            ident = const_pool.tile([64, 64], F32)
            nc.scalar.dma_start(ident[:], ident_d.ap())
            identb = const_pool.tile([128, 128], BF16, tag="identb")
            nc.scalar.dma_start(identb[:], identb_d.ap())
            ones = const_pool.tile([128, 128], BF16)
            nc.vector.memset(ones[:], 1.0)
            w_c_t = const_pool.tile([128, 2], F32, tag="w_c_t")
            w_cq_t = const_pool.tile([128, 2], F32, tag="w_cq_t")
            for w_t, w_dram in ((w_c_t, w_c_d), (w_cq_t, w_cq_d)):
                nc.scalar.dma_start(w_t[:], w_dram.ap().rearrange("(k p) -> p k", p=128))
            w_q_b = const_pool.tile([128, 2], BF16, tag="w_q_b")
            nc.gpsimd.dma_start(w_q_b[:], w_q_d.ap().rearrange("(k p) -> p k", p=128))
            b3 = const_pool.tile([1, 3], F32, tag="b3")
            for i, b_dram in enumerate((b_c_d, b_q_d, b_cq_d)):
                nc.scalar.dma_start(b3[:, i : i + 1], b_dram.ap().unsqueeze(0))
            bsum = const_pool.tile([1, 1], F32, tag="bsum")
            nc.vector.reduce_sum(bsum[:], b3[:], axis=mybir.AxisListType.X)

            # ---- q-side prep for all batch elems first ----
            Rs, sq_rows, Us, qc_pss = [], [], [], []
            for n in range(N_per):
                q_sb = qprep.tile([QL, D], F32, tag="q_sb")
                nc.scalar.dma_start(q_sb[:], q_d.ap()[n])
                qt_ps = tpsum_pool.tile([128, 2, QL], F32, tag="tp")
                for k in range(2):
                    nc.tensor.transpose(
                        qt_ps[:, k, :], q_sb[:, 128 * k : 128 * (k + 1)], ident[:]
                    )
                qt_b16 = qprep.tile([128, 2, QL], BF16, tag="qt_b16")
                R = qprep.tile([128, 2, QL], BF16, tag="R")
                for k in range(2):
                    nc.scalar.copy(qt_b16[:, k, :], qt_ps[:, k, :])
                    nc.scalar.activation(
                        R[:, k, :],
                        qt_ps[:, k, :],
                        mybir.ActivationFunctionType.Identity,
                        bias=w_c_t[:, k : k + 1],
                        scale=w_cq_t[:, k : k + 1],
                    )
                qc_ps = qc_pool.tile([1, 512], F32)
                for k in range(2):
                    nc.tensor.matmul(
                        qc_ps[:, 320 : 320 + QL],
                        w_q_b[:, k : k + 1],
                        qt_b16[:, k, :],
                        start=(k == 0),
                        stop=(k == 1),
                        skip_group_check=True,
                    )
                sq_row = qprep.tile([1, QL], BF16, tag="sq_row")
                nc.scalar.activation(
                    sq_row[:],
                    qc_ps[0:1, 320 : 320 + QL],
                    mybir.ActivationFunctionType.Identity,
                    bias=bsum[:, 0:1],
                )
                U = u_pool.tile([128, T], BF16)
                Rs.append(R); sq_rows.append(sq_row); Us.append(U); qc_pss.append(qc_ps)

            # ---- main loop: interleave the two batch elems group by group ----
            def _emit_q2c(item):
                cbp, Up, gp = item
                n_of = Us.index(Up)
                for t in range(GROUP):
                    blk, r = t // 2, t % 2
                    gt = gp * GROUP + t
                    nc.tensor.matmul(
                        qc_pss[n_of][:, 0:D],
                        Up[:, gt : gt + 1],
                        cbp[:, blk, r, :],
                        start=(gt == 0),
                        stop=(gt == T - 1),
                        skip_group_check=True,
                    )

            prev_q2c = []
            for gi in range(N_per * G):
                n, g = gi % N_per, gi // N_per
                R, sq_row, U, qc_ps = Rs[n], sq_rows[n], Us[n], qc_pss[n]
                lo = g * GROUP * 128
                hi = (g + 1) * GROUP * 128
                # load 8 cl-tiles, casting f32 -> bf16 inline (SWDGE)
                cb = cb_pool.tile([128, GROUP // 2, 2, D], BF16)
                nc.gpsimd.dma_start(
                    cb[:],
                    c_d.ap()[n, lo:hi, :].rearrange(
                        "(blk p r) d -> p blk r d", p=128, r=2
                    ),
                )
                # PE transpose: cb [128cl, (t,d)] -> cbT chunks [128d, 128cl]
                cbT = cbT_pool.tile([128, 2 * GROUP, 128], BF16)
                for cc in range(2):
                    tps = tpsum_pool.tile([128, GROUP, 128], BF16, tag="tp")
                    for j in range(GROUP):
                        jj = cc * GROUP + j      # chunk index 0..15
                        blk, r, k = jj // 4, (jj // 2) % 2, jj % 2
                        nc.tensor.transpose(
                            tps[:, j, :],
                            cb[:, blk, r, 128 * k : 128 * (k + 1)],
                            identb[:],
                        )
                    if cc == 0:
                        nc.scalar.copy(
                            cbT[:, cc * GROUP : (cc + 1) * GROUP, :], tps[:]
                        )
                    else:
                        nc.vector.tensor_copy(
                            cbT[:, cc * GROUP : (cc + 1) * GROUP, :], tps[:]
                        )
                # s = ones^T @ (sq_row + bsum broadcast)  +  c @ R
                sps = spsum_pool.tile([128, GROUP, QL], F32)
                nc.tensor.matmul(
                    sps[:],
                    ones[0:1, :],
                    sq_row[:].unsqueeze(1).broadcast_to((1, GROUP, QL)),
                    start=True,
                    stop=False,
                    skip_group_check=True,
                )
                for t in range(GROUP):
                    for k in range(2):
                        nc.tensor.matmul(
                            sps[:, t, :],
                            cbT[:, 2 * t + k, :],
                            R[:, k, :],
                            start=False,
                            stop=(k == 1),
                            skip_group_check=True,
                        )
                prev_q2c.append((cb, n, g))
                prev_q2c.append((cb, U, g))
                # softmax branch first (it recycles cb via the q2c matmuls)
                mg = m_pool.tile([128, GROUP], F32)
                nc.vector.reduce_max(mg[:], sps[:], axis=mybir.AxisListType.X)
                nc.scalar.activation(
                    U[:, g * GROUP : (g + 1) * GROUP],
                    mg[:],
                    mybir.ActivationFunctionType.Exp,
                )
                if len(prev_q2c) > 1:
                    _emit_q2c(prev_q2c.pop(0))
                # s epilogue + store
                s_sb = ssb_pool.tile([128, GROUP // 2, 2, QL], F32)
                nc.scalar.copy(
                    s_sb[:].rearrange("p blk r q -> p (blk r) q"), sps[:]
                )
                nc.sync.dma_start(
                    s_d.ap()[n, lo:hi, :].rearrange(
                        "(blk p r) q -> p blk (r q)", p=128, r=2
                    ),
                    s_sb[:].rearrange("p blk r q -> p blk (r q)"),
                )

            while prev_q2c:
                _emit_q2c(prev_q2c.pop(0))

            while prev_q2c:
                _emit_q2c(prev_q2c.pop(0))

            for n in range(N_per):
                U, qc_ps = Us[n], qc_pss[n]
                # Z = sum(U) via ones^T @ U then a free-dim reduce; q2c = qc / Z
                nc.tensor.matmul(
                    qc_ps[:, D : D + T],
                    ones[:, 0:1],
                    U[:],
                    start=True,
                    stop=True,
                    skip_group_check=True,
                )
                zz = qprep.tile([1, 1], F32, tag="zz")
                nc.vector.reduce_sum(
                    zz[:], qc_ps[0:1, D : D + T], axis=mybir.AxisListType.X
                )
                rz = qprep.tile([1, 1], F32, tag="rz")
                nc.vector.reciprocal(rz[:], zz[:])
                q2c_sb = out_pool.tile([1, D], F32)
                nc.scalar.activation(
                    q2c_sb[:],
                    qc_ps[0:1, 0:D],
                    mybir.ActivationFunctionType.Copy,
                    bias=0.0,
                    scale=rz[0:1, 0:1],
                )
                nc.scalar.dma_start(q2c_d.ap()[n].unsqueeze(0), q2c_sb[:])

    nc.compile()
    return nc


_NC_CACHE = {}


def _get_nc(key):
    if key not in _NC_CACHE:
        _NC_CACHE[key] = build_kernel(*key)
    return _NC_CACHE[key]


def kernel(c, q, w_c, b_c, w_q, b_q, w_cq, b_cq, **run_kwargs):
    N, CL, D = c.shape
    _, QL, _ = q.shape
    n_per = N // N_CORES
    nc = _get_nc((n_per, CL, QL, D))

    import ml_dtypes

    ident = np.eye(64, dtype=np.float32)
    identb = np.eye(128, dtype=np.float32).astype(ml_dtypes.bfloat16)
    in_maps = []
    for i in range(N_CORES):
        sl = slice(i * n_per, (i + 1) * n_per)
        in_maps.append(
            {
                "c": np.ascontiguousarray(c[sl], dtype=np.float32),
                "q": np.ascontiguousarray(q[sl], dtype=np.float32),
                "w_c": np.asarray(w_c, dtype=np.float32),
                "w_q": np.asarray(w_q, dtype=np.float32),
                "w_cq": np.asarray(w_cq, dtype=np.float32),
                "b_c": np.asarray(b_c, dtype=np.float32),
                "b_q": np.asarray(b_q, dtype=np.float32),
                "b_cq": np.asarray(b_cq, dtype=np.float32),
                "ident": ident,
                "identb": identb,
            }
        )

    res = run_bass_kernel_spmd(
        nc, in_maps, core_ids=list(range(N_CORES)), **run_kwargs
    )
    q2c = np.concatenate([res.results[i]["q2c_out"] for i in range(N_CORES)], axis=0)
    s = np.concatenate([res.results[i]["s_out"] for i in range(N_CORES)], axis=0)
    if run_kwargs:
        return (q2c, s), res
    return q2c, s


# revision 19
# speedup vs baseline: 2.1436x; 1.0330x over previous
"""BiDAF attention-flow kernel for Trainium2, data-parallel over batch on 8 cores.

Reference computation (per batch element n):
    s_c = c @ w_c + b_c                       # [CL]
    s_q = q @ w_q + b_q                       # [QL]
    cq  = (c * w_cq) @ q^T + b_cq             # [CL, QL]
    s   = cq + s_c[:, None] + s_q[None, :]    # [CL, QL]
    b_att = softmax(max(s, axis=1))           # [CL]
    q2c  = b_att @ c                          # [D]
    return q2c, s

Device mapping (per core, 2 batch elements):
  - R[d, q] = w_cq[d] * q[n, q, d] + w_c[d] folds the s_c term into the GEMM
    (sum_d c[cl,d] * w_c[d] = s_c[cl]); the s_q + b terms enter via a K=1
    ones-row matmul that pre-fills each PSUM bank.
  - c is DMA-loaded with an inline f32->bf16 cast (SWDGE), transposed on-chip
    by the DMA xbar (one [128, 2048] -> [128, 16, 128] transpose per 8-tile
    group) to give d-major lhsT chunks for the TensorE GEMM.
  - softmax skips the max-subtraction (values are O(10), exp is safe in f32)
    so q2c accumulates per-tile with no global barrier:
    q2c = (sum_cl exp(m_cl) * c[cl,:]) / sum_cl exp(m_cl).
"""

import numpy as np

import concourse.bacc as bacc
import concourse.mybir as mybir
import concourse.tile as tile
from concourse.bass_utils import run_bass_kernel_spmd

F32 = mybir.dt.float32
F32R = mybir.dt.float32r
BF16 = mybir.dt.bfloat16

N_CORES = 8
N_FULL, CL_FULL, QL_FULL, D_FULL = 16, 4096, 64, 256


def build_kernel(N_per=2, CL=4096, QL=64, D=256, GROUP=8):
    """Build the bass program for one core processing N_per batch elements."""
    assert D == 256 and QL == 64
    T = CL // 128            # cl tiles per batch elem
    G = T // GROUP           # tile groups per batch elem
    assert G * GROUP == T

    nc = bacc.Bacc(
        "TRN2",
        target_bir_lowering=False,
        debug=False,
        dynamic_dma_scratch_size=65536,
        num_swdge_queues=2,
        num_swdge_queues=2,
    )

    c_d = nc.dram_tensor("c", [N_per, CL, D], F32, kind="ExternalInput")
    q_d = nc.dram_tensor("q", [N_per, QL, D], F32, kind="ExternalInput")
    w_c_d = nc.dram_tensor("w_c", [D], F32, kind="ExternalInput")
    w_q_d = nc.dram_tensor("w_q", [D], F32, kind="ExternalInput")
    w_cq_d = nc.dram_tensor("w_cq", [D], F32, kind="ExternalInput")
    b_c_d = nc.dram_tensor("b_c", [1], F32, kind="ExternalInput")
    b_q_d = nc.dram_tensor("b_q", [1], F32, kind="ExternalInput")
    b_cq_d = nc.dram_tensor("b_cq", [1], F32, kind="ExternalInput")
    ident_d = nc.dram_tensor("ident", [64, 64], F32, kind="ExternalInput")
    identb_d = nc.dram_tensor("identb", [128, 128], BF16, kind="ExternalInput")

    s_d = nc.dram_tensor("s_out", [N_per, CL, QL], F32, kind="ExternalOutput")
    q2c_d = nc.dram_tensor("q2c_out", [N_per, D], F32, kind="ExternalOutput")

    with tile.TileContext(nc) as tc:
        with (
            tc.tile_pool(name="const", bufs=1) as const_pool,
            tc.tile_pool(name="qprep", bufs=2) as qprep,
                        tc.tile_pool(name="cb", bufs=8) as cb_pool,
            tc.tile_pool(name="cbT", bufs=4) as cbT_pool,
            tc.tile_pool(name="spsum", bufs=3, space="PSUM") as spsum_pool,
            tc.tile_pool(name="tpsum", bufs=3, space="PSUM") as tpsum_pool,
            tc.tile_pool(name="ssb", bufs=4) as ssb_pool,
            tc.tile_pool(name="mg", bufs=3) as m_pool,
            tc.tile_pool(name="upool", bufs=2) as u_pool,
            tc.tile_pool(name="qcpsum", bufs=2, space="PSUM") as qc_pool,
            tc.tile_pool(name="outsb", bufs=2) as out_pool,
        ):
            # ---- constants ----
            ident = const_pool.tile([64, 64], F32)
            nc.scalar.dma_start(ident[:], ident_d.ap())
            identb = const_pool.tile([128, 128], BF16, tag="identb")
            nc.scalar.dma_start(identb[:], identb_d.ap())
            ones = const_pool.tile([128, 128], BF16)
            nc.vector.memset(ones[:], 1.0)
            w_c_t = const_pool.tile([128, 2], F32, tag="w_c_t")
            w_cq_t = const_pool.tile([128, 2], F32, tag="w_cq_t")
            for w_t, w_dram in ((w_c_t, w_c_d), (w_cq_t, w_cq_d)):
                nc.scalar.dma_start(w_t[:], w_dram.ap().rearrange("(k p) -> p k", p=128))
            w_q_b = const_pool.tile([128, 2], BF16, tag="w_q_b")
            b3 = const_pool.tile([1, 3], F32, tag="b3")
            for i, b_dram in enumerate((b_c_d, b_q_d, b_cq_d)):
                nc.scalar.dma_start(b3[:, i : i + 1], b_dram.ap().unsqueeze(0))
            bsum = const_pool.tile([1, 1], F32, tag="bsum")
            nc.vector.reduce_sum(bsum[:], b3[:], axis=mybir.AxisListType.X)

            # ---- q-side prep for all batch elems first ----
            nc.gpsimd.dma_start(w_q_b[:], w_q_d.ap().rearrange("(k p) -> p k", p=128))
            Rs, sq_rows, Us, qc_pss = [], [], [], []
            for n in range(N_per):
                q_sb = qprep.tile([QL, D], F32, tag="q_sb")
                nc.scalar.dma_start(q_sb[:], q_d.ap()[n])
                qt_ps = tpsum_pool.tile([128, 2, QL], F32, tag="tp")
                for k in range(2):
                    nc.tensor.transpose(
                        qt_ps[:, k, :], q_sb[:, 128 * k : 128 * (k + 1)], ident[:]
                    )
                qt_b16 = qprep.tile([128, 2, QL], BF16, tag="qt_b16")
                R = qprep.tile([128, 2, QL], BF16, tag="R")
                for k in range(2):
                    nc.scalar.copy(qt_b16[:, k, :], qt_ps[:, k, :])
                    nc.scalar.activation(
                        R[:, k, :],
                        qt_ps[:, k, :],
                        mybir.ActivationFunctionType.Identity,
                        bias=w_c_t[:, k : k + 1],
                        scale=w_cq_t[:, k : k + 1],
                    )
                qc_ps = qc_pool.tile([1, 512], F32)
                for k in range(2):
                    nc.tensor.matmul(
                        qc_ps[:, 320 : 320 + QL],
                        w_q_b[:, k : k + 1],
                        qt_b16[:, k, :],
                        start=(k == 0),
                        stop=(k == 1),
                        skip_group_check=True,
                    )
                sq_row = qprep.tile([1, QL], BF16, tag="sq_row")
                nc.scalar.activation(
                    sq_row[:],
                    qc_ps[0:1, 320 : 320 + QL],
                    mybir.ActivationFunctionType.Identity,
                    bias=bsum[:, 0:1],
                )
                U = u_pool.tile([128, T], BF16)
                Rs.append(R); sq_rows.append(sq_row); Us.append(U); qc_pss.append(qc_ps)

            # ---- main loop: interleave the two batch elems group by group ----
            def _emit_q2c(item):
                cbp, Up, gp = item
                n_of = Us.index(Up)
                for t in range(GROUP):
                    blk, r = t // 2, t % 2
                    gt = gp * GROUP + t
                    nc.tensor.matmul(
                        qc_pss[n_of][:, 0:D],
                        Up[:, gt : gt + 1],
                        cbp[:, blk, r, :],
                        start=(gt == 0),
                        stop=(gt == T - 1),
                        skip_group_check=True,
                    )

            prev_q2c = []
            for gi in range(N_per * G):
                n, g = gi % N_per, gi // N_per
                R, sq_row, U, qc_ps = Rs[n], sq_rows[n], Us[n], qc_pss[n]
                lo = g * GROUP * 128
                hi = (g + 1) * GROUP * 128
                # load 8 cl-tiles, casting f32 -> bf16 inline (SWDGE)
                cb = cb_pool.tile([128, GROUP // 2, 2, D], BF16)
                nc.gpsimd.dma_start(
                    cb[:],
                    c_d.ap()[n, lo:hi, :].rearrange(
                        "(blk p r) d -> p blk r d", p=128, r=2
                    ),
                )
                # PE transpose: cb [128cl, (t,d)] -> cbT chunks [128d, 128cl]
                cbT = cbT_pool.tile([128, 2 * GROUP, 128], BF16)
                for cc in range(2):
                    tps = tpsum_pool.tile([128, GROUP, 128], BF16, tag="tp")
                    for j in range(GROUP):
                        jj = cc * GROUP + j      # chunk index 0..15
                        blk, r, k = jj // 4, (jj // 2) % 2, jj % 2
                        nc.tensor.transpose(
                            tps[:, j, :],
                            cb[:, blk, r, 128 * k : 128 * (k + 1)],
                            identb[:],
                        )
                    if cc == 0:
                        nc.scalar.copy(
                            cbT[:, cc * GROUP : (cc + 1) * GROUP, :], tps[:]
                        )
                    else:
                        nc.vector.tensor_copy(
                            cbT[:, cc * GROUP : (cc + 1) * GROUP, :], tps[:]
                        )
                # s = ones^T @ (sq_row + bsum broadcast)  +  c @ R
                sps = spsum_pool.tile([128, GROUP, QL], F32)
                nc.tensor.matmul(
                    sps[:],
                    ones[0:1, :],
                    sq_row[:].unsqueeze(1).broadcast_to((1, GROUP, QL)),
                    start=True,
                    stop=False,
                    skip_group_check=True,
                )
                for t in range(GROUP):
                    for k in range(2):
                        nc.tensor.matmul(
                            sps[:, t, :],
                            cbT[:, 2 * t + k, :],
                            R[:, k, :],
                            start=False,
                            stop=(k == 1),
                            skip_group_check=True,
                        )
                prev_q2c.append((cb, n, g))
                prev_q2c.append((cb, U, g))
                # softmax branch first (it recycles cb via the q2c matmuls)
                mg = m_pool.tile([128, GROUP], F32)
                nc.vector.reduce_max(mg[:], sps[:], axis=mybir.AxisListType.X)
                nc.scalar.activation(
                    U[:, g * GROUP : (g + 1) * GROUP],
                    mg[:],
                    mybir.ActivationFunctionType.Exp,
                )
                if len(prev_q2c) > 1:
                    _emit_q2c(prev_q2c.pop(0))
                # s epilogue + store
                s_sb = ssb_pool.tile([128, GROUP // 2, 2, QL], F32)
                if gi % 2 == 0:
                    nc.scalar.copy(
                        s_sb[:].rearrange("p blk r q -> p (blk r) q"), sps[:]
                    )
                else:
                    nc.vector.tensor_copy(
                        s_sb[:].rearrange("p blk r q -> p (blk r) q"), sps[:]
                    )
                nc.sync.dma_start(
                    s_d.ap()[n, lo:hi, :].rearrange(
                        "(blk p r) q -> p blk (r q)", p=128, r=2
                    ),
                    s_sb[:].rearrange("p blk r q -> p blk (r q)"),
                )

            while prev_q2c:
                _emit_q2c(prev_q2c.pop(0))

            while prev_q2c:
                _emit_q2c(prev_q2c.pop(0))

            for n in range(N_per):
                U, qc_ps = Us[n], qc_pss[n]
                # Z = sum(U) via ones^T @ U then a free-dim reduce; q2c = qc / Z
                nc.tensor.matmul(
                    qc_ps[:, D : D + T],
                    ones[:, 0:1],
                    U[:],
                    start=True,
                    stop=True,
                    skip_group_check=True,
                )
                zz = qprep.tile([1, 1], F32, tag="zz")
                nc.vector.reduce_sum(
                    zz[:], qc_ps[0:1, D : D + T], axis=mybir.AxisListType.X
                )
                rz = qprep.tile([1, 1], F32, tag="rz")
                nc.vector.reciprocal(rz[:], zz[:])
                q2c_sb = out_pool.tile([1, D], F32)
                nc.scalar.activation(
                    q2c_sb[:],
                    qc_ps[0:1, 0:D],
                    mybir.ActivationFunctionType.Copy,
                    bias=0.0,
                    scale=rz[0:1, 0:1],
                )
                nc.scalar.dma_start(q2c_d.ap()[n].unsqueeze(0), q2c_sb[:])

    nc.compile()
    return nc


_NC_CACHE = {}


def _get_nc(key):
    if key not in _NC_CACHE:
        _NC_CACHE[key] = build_kernel(*key)
    return _NC_CACHE[key]


def kernel(c, q, w_c, b_c, w_q, b_q, w_cq, b_cq, **run_kwargs):
    N, CL, D = c.shape
    _, QL, _ = q.shape
    n_per = N // N_CORES
    nc = _get_nc((n_per, CL, QL, D))

    import ml_dtypes

    ident = np.eye(64, dtype=np.float32)
    identb = np.eye(128, dtype=np.float32).astype(ml_dtypes.bfloat16)
    in_maps = []
    for i in range(N_CORES):
        sl = slice(i * n_per, (i + 1) * n_per)
        in_maps.append(
            {
                "c": np.ascontiguousarray(c[sl], dtype=np.float32),
                "q": np.ascontiguousarray(q[sl], dtype=np.float32),
                "w_c": np.asarray(w_c, dtype=np.float32),
                "w_q": np.asarray(w_q, dtype=np.float32),
                "w_cq": np.asarray(w_cq, dtype=np.float32),
                "b_c": np.asarray(b_c, dtype=np.float32),
                "b_q": np.asarray(b_q, dtype=np.float32),
                "b_cq": np.asarray(b_cq, dtype=np.float32),
                "ident": ident,
                "identb": identb,
            }
        )

    res = run_bass_kernel_spmd(
        nc, in_maps, core_ids=list(range(N_CORES)), **run_kwargs
    )
    q2c = np.concatenate([res.results[i]["q2c_out"] for i in range(N_CORES)], axis=0)
    s = np.concatenate([res.results[i]["s_out"] for i in range(N_CORES)], axis=0)
    if run_kwargs:
        return (q2c, s), res
    return q2c, s
